# revision 14
# baseline (speedup 1.0000x reference)
"""Multi-headed self-attention (B=8, S=1024, D=768, H=12) on 8 TRN2 cores.

Sharding: data-parallel over batch -- core i computes batch element i.
Per-core kernel (all operands pre-transposed on host):
    Qt = (Wq @ x.T + bq)      [D, S]   (o on partitions)
    Kt = (Wk @ x.T + bk)      [D, S]
    V  = (x @ Wv.T + bv)      [S, D]   augmented with a ones column per head
    St_h = Kt_h^T-slices @ Qt_h   -> scores transposed [k, q]
    Et = exp(St/8 + maskbias[k])  (ACT, mask bias per-partition)
    PVt'_h = V'_h.T @ Et_h        [65, q]; row 64 = sum_k Et = Z[q]
    out_h.T = PVt'_h[0:64] / Z    -> outT rows h*64..h*64+63
Host transposes outT back.
"""

import numpy as np

import concourse.bacc as bacc
import concourse.tile as tile
from concourse import mybir
from concourse.bass_utils import run_bass_kernel_spmd

B, S, D, H = 8, 1024, 768, 12
HD = D // H  # 64
N_CORES = 8
SC = S // 128  # 8 key/seq chunks
OC = D // 128  # 6 output chunks (2 heads each)
DC = D // 128  # 6 contraction chunks
NT = 512  # matmul moving-dim tile (fp32 max)
QT = S // NT  # 2
F32 = mybir.dt.float32
F32R = mybir.dt.float32r

HW = HD + 1  # per-head V width incl. ones column


def build():
    nc = bacc.Bacc("TRN2", target_bir_lowering=False, debug=False, num_devices=N_CORES)
    xT = nc.dram_tensor("xT", [D, S], F32R, kind="ExternalInput").ap()
    wqT = nc.dram_tensor("wqT", [D, D], F32R, kind="ExternalInput").ap()
    wkT = nc.dram_tensor("wkT", [D, D], F32R, kind="ExternalInput").ap()
    wvT = nc.dram_tensor("wvT", [D, D], F32R, kind="ExternalInput").ap()
    bq = nc.dram_tensor("bq", [D], F32, kind="ExternalInput").ap()
    bk = nc.dram_tensor("bk", [D], F32, kind="ExternalInput").ap()
    bvb = nc.dram_tensor("bvb", [128, D], F32, kind="ExternalInput").ap()
    mb = nc.dram_tensor("mb", [S], F32, kind="ExternalInput").ap()
    outT = nc.dram_tensor("outT", [D, S], F32, kind="ExternalOutput").ap()

    with tile.TileContext(nc) as tc:
        with (
            tc.tile_pool(name="const", bufs=1) as const,
            tc.tile_pool(name="qk", bufs=2) as qk_pool,
            tc.tile_pool(name="et", bufs=8) as et_pool,
            tc.tile_pool(name="epi", bufs=2) as epi_pool,
            tc.tile_pool(name="st", bufs=3, space="PSUM") as st_ps,
            tc.tile_pool(name="pv", bufs=2, space="PSUM") as pv_ps,
            tc.tile_pool(name="dram", bufs=2, space="DRAM") as dram_pool,
        ):
            # ---------- constant / weight loads ----------
            xt = [const.tile([128, S], F32R, tag=f"xt{c}", name=f"xt{c}") for c in range(DC)]
            wq = [const.tile([128, D], F32R, tag=f"wq{c}", name=f"wq{c}") for c in range(DC)]
            wk = [const.tile([128, D], F32R, tag=f"wk{c}", name=f"wk{c}") for c in range(DC)]
            wv = [const.tile([128, D], F32R, tag=f"wv{c}", name=f"wv{c}") for c in range(DC)]
            # V-projection inputs first (it runs first), then Q/K weights
            for c in range(DC):
                nc.sync.dma_start(xt[c][:], xT[c * 128:(c + 1) * 128, :])
                nc.sync.dma_start(wv[c][:], wvT[c * 128:(c + 1) * 128, :])
            for c in range(DC):
                nc.sync.dma_start(wq[c][:], wqT[c * 128:(c + 1) * 128, :])
                nc.sync.dma_start(wk[c][:], wkT[c * 128:(c + 1) * 128, :])
            bq_t = const.tile([128, OC], F32, tag="bq")
            nc.sync.dma_start(bq_t[:], bq.rearrange("(c p) -> p c", p=128))
            bk_t = const.tile([128, OC], F32, tag="bk")
            nc.sync.dma_start(bk_t[:], bk.rearrange("(c p) -> p c", p=128))
            bvb_t = const.tile([128, D], F32, tag="bvb")
            nc.sync.dma_start(bvb_t[:], bvb[:])
            mb_t = const.tile([128, SC], F32, tag="mb")
            nc.sync.dma_start(mb_t[:], mb.rearrange("(c p) -> p c", p=128))

            # ---------- V projection -> vaug [sc][128, H*65] ----------
            vaug = [const.tile([128, H * HW], F32R, tag=f"va{sc}", name=f"va{sc}") for sc in range(SC)]
            for sc in range(SC):
                ones_cols = vaug[sc][:].rearrange("p (h w) -> p h w", h=H)[:, :, HD:HW]
                nc.vector.memset(ones_cols.bitcast(F32), 1.0)
            for sc in range(SC):
                for n0, n1, h0, h1 in ((0, 512, 0, 8), (512, 768, 8, 12)):
                    vp = st_ps.tile([128, NT], F32, tag="st", name="vp")
                    for c in range(DC):
                        nc.tensor.matmul(
                            vp[:, : n1 - n0],
                            xt[c][:, sc * 128:(sc + 1) * 128],
                            wv[c][:, n0:n1],
                            start=(c == 0),
                            stop=(c == DC - 1),
                        )
                    nc.vector.tensor_add(
                        vaug[sc][:].rearrange("p (h w) -> p h w", h=H)[:, h0:h1, 0:HD],
                        vp[:, : n1 - n0].rearrange("p (h w) -> p h w", w=HD),
                        bvb_t[:, n0:n1].rearrange("p (h w) -> p h w", w=HD),
                    )

            # ---------- Q/K projection for one o-chunk ----------
            def qk_proj(oc):
                out = {}
                for name, w_t, b_t in (("q", wq, bq_t), ("k", wk, bk_t)):
                    dst = qk_pool.tile([128, S], F32R, tag=name, name=f"{name}t{oc}")
                    for qt in range(QT):
                        p = st_ps.tile([128, NT], F32, tag="st", name="qkp")
                        for c in range(DC):
                            nc.tensor.matmul(
                                p[:],
                                w_t[c][:, oc * 128:(oc + 1) * 128],
                                xt[c][:, qt * NT:(qt + 1) * NT],
                                start=(c == 0),
                                stop=(c == DC - 1),
                            )
                        nc.vector.tensor_scalar_add(
                            dst[:, qt * NT:(qt + 1) * NT], p[:], b_t[:, oc:oc + 1]
                        )
                    out[name] = dst
                return out

            # ---------- attention: flat software pipeline, skew=2 ----------
            # PE stream per unit i: [scores(i+SKEW), pv(i)] so the PE always
            # has slot-ready scores work while pv(i) waits on exp(i).
            qkts = {0: qk_proj(0)}
            units = [(oc, hh, kc) for oc in range(OC) for hh in range(2)
                     for kc in range(SC)]
            NU = len(units)
            SKEW = 2
            st_tiles = {}
            pvq_map = {}

            def emit_scores(i):
                oc, hh, kc = units[i]
                p0 = hh * 64
                qkt = qkts[oc]
                stt = st_ps.tile([128, S], F32, tag="st", name=f"st{i}")
                for qt in range(QT):
                    nc.tensor.matmul(
                        stt[:, qt * NT:(qt + 1) * NT],
                        qkt["k"][p0:p0 + 64, kc * 128:(kc + 1) * 128],
                        qkt["q"][p0:p0 + 64, qt * NT:(qt + 1) * NT],
                        tile_position=(p0, 0),
                    )
                st_tiles[i] = stt

            def emit_epilogue(oc, hh):
                gh = 2 * oc + hh
                pvq = pvq_map.pop((oc, hh))
                pvs = epi_pool.tile([HW, S], F32, tag="pvs", name="pvs", bufs=3)
                for qt in range(QT):
                    nc.vector.tensor_copy(
                        pvs[:, qt * NT:(qt + 1) * NT], pvq[qt][:]
                    )
                # Z row -> [128, 8] partition-scatter (p-major), reciprocal,
                # bounce through DRAM for the partition-broadcast read.
                zp = epi_pool.tile([128, SC], F32, tag="zp", name="zp", bufs=4)
                nc.gpsimd.dma_start(
                    zp[:], pvs[HD:HW, :].rearrange("o (p c) -> o p c", c=SC)
                )
                nc.vector.reciprocal(zp[:], zp[:])
                rzd = dram_pool.tile([S], F32, tag="rzd", name="rzd", bufs=4)
                nc.gpsimd.dma_start(rzd.rearrange("(p c) -> p c", c=SC), zp[:])
                zb = epi_pool.tile([HD, S], F32, tag="zb", name="zb", bufs=3)
                nc.gpsimd.dma_start(zb[:], rzd[:].partition_broadcast(HD))
                oh = epi_pool.tile([HD, S], F32, tag="oh", name="oh", bufs=3)
                nc.vector.tensor_mul(oh[:], pvs[0:HD, :], zb[:])
                nc.sync.dma_start(outT[gh * HD:(gh + 1) * HD, :], oh[:])

            for i in range(SKEW):
                emit_scores(i)
            for i, (oc, hh, kc) in enumerate(units):
                if i + SKEW < NU:
                    emit_scores(i + SKEW)
                stt = st_tiles.pop(i)
                ett = et_pool.tile([128, S], F32R, tag="et", name=f"et{i}")
                nc.scalar.activation(
                    ett[:],
                    stt[:],
                    mybir.ActivationFunctionType.Exp,
                    bias=mb_t[:, kc:kc + 1],
                    scale=1.0 / np.sqrt(HD),
                )
                gh = 2 * oc + hh
                if kc == 0:
                    pvq_map[(oc, hh)] = [
                        pv_ps.tile([HW, NT], F32, tag="pv", name=f"pv{gh}_{qt}")
                        for qt in range(QT)
                    ]
                pvq = pvq_map[(oc, hh)]
                for qt in range(QT):
                    nc.tensor.matmul(
                        pvq[qt][:],
                        vaug[kc][:, gh * HW:(gh + 1) * HW],
                        ett[:, qt * NT:(qt + 1) * NT],
                        start=(kc == 0),
                        stop=(kc == SC - 1),
                    )
                if kc == SC - 1:
                    emit_epilogue(oc, hh)
                if hh == 1 and kc == 3 and oc + 1 < OC:
                    qkts[oc + 1] = qk_proj(oc + 1)
                    qkts.pop(oc - 1, None)

    nc.compile()
    return nc


_NC = None


def _get_nc():
    global _NC
    if _NC is None:
        _NC = build()
    return _NC


def _in_maps(x, mask, Wq, bq, Wk, bk, Wv, bv):
    x = np.asarray(x, dtype=np.float32)
    mask = np.asarray(mask)
    wqT = np.ascontiguousarray(np.asarray(Wq, dtype=np.float32).T)
    wkT = np.ascontiguousarray(np.asarray(Wk, dtype=np.float32).T)
    wvT = np.ascontiguousarray(np.asarray(Wv, dtype=np.float32).T)
    bq = np.asarray(bq, dtype=np.float32)
    bk = np.asarray(bk, dtype=np.float32)
    bvb = np.ascontiguousarray(
        np.broadcast_to(np.asarray(bv, dtype=np.float32), (128, D))
    )
    maps = []
    for c in range(N_CORES):
        maps.append(
            {
                "xT": np.ascontiguousarray(x[c].T),
                "wqT": wqT,
                "wkT": wkT,
                "wvT": wvT,
                "bq": bq,
                "bk": bk,
                "bvb": bvb,
                "mb": (-10000.0 * (1.0 - mask[c].astype(np.float32))).astype(
                    np.float32
                ),
            }
        )
    return maps


def run(inputs, trace=False, **kw):
    nc = _get_nc()
    res = run_bass_kernel_spmd(
        nc, _in_maps(**inputs), list(range(N_CORES)), trace=trace, **kw
    )
    out = np.stack(
        [np.ascontiguousarray(res.results[c]["outT"].T) for c in range(N_CORES)]
    ).astype(np.float32)
    return out, res


def kernel(**inputs):
    out, _ = run(inputs)
    return out
